# revision 53
# baseline (speedup 1.0000x reference)
"""Multi-head attention (B=2, S=2048, H=16, HD=64, D=1024) on 8 trn2 cores.

Sharding: 2 heads per core (tensor-parallel over heads). Each core computes
its heads' Q/K/V projections (column-sharded weights), full attention for its
4 (batch, head) pairs, and a partial output projection (row-sharded Wo).
Host sums the 8 partials and adds bo.

The scalar engine's exp is the hard floor (1 elem/cycle/partition ->
~143us/core for the 16.8M scores), so the kernel is built as one continuous
exp pipeline: a global scores->exp cursor runs 2 k-blocks ahead across chunk
boundaries, attn@V consumes exp pairs as fp8 DoubleRow matmuls (K=256,
2 rows/cycle), and all projection / output-projection work is drained as
micro-tasks in the per-iteration PE slack. Batch-0 chunk-0 attention is woven
into the projection prologue so exp starts as early as possible.
"""
import os
from collections import deque
from contextlib import ExitStack

import numpy as np
import ml_dtypes

import concourse.bass as bass
import concourse.tile as tile
import concourse.mybir as mybir
from concourse import bacc
from concourse.bass_utils import run_bass_kernel_spmd
from concourse.masks import make_identity

B, S, D = 2, 2048, 1024
H, HD = 16, 64
NCORES = 8
HPC = H // NCORES          # heads per core = 2
CW = HPC * HD              # column width per core = 128
R = B * S                  # total rows = 4096
NKB = S // 128             # k-blocks per (b,h) = 16
NQ = S // 512              # q-chunks per batch = 4
NC8 = D // 128             # d_in chunks = 8
NRB = R // 512             # 512-row projection blocks = 8
NT = B * NQ * NKB          # global k-block count = 128

F32 = mybir.dt.float32
BF16 = mybir.dt.bfloat16
FP8 = mybir.dt.float8e4
DRM = mybir.MatmulPerfMode.DoubleRow
AF = mybir.ActivationFunctionType


def build():
    nc = bacc.Bacc("TRN2", target_bir_lowering=False, debug=False)
    xT = nc.dram_tensor("xT", [D, R], BF16, kind="ExternalInput")
    # weights pre-transposed on host to [128, NC8*CW] (partition-major)
    Wq = nc.dram_tensor("Wq", [128, NC8 * CW], BF16, kind="ExternalInput")
    Wk = nc.dram_tensor("Wk", [128, NC8 * CW], BF16, kind="ExternalInput")
    Wv = nc.dram_tensor("Wv", [128, NC8 * CW], BF16, kind="ExternalInput")
    bq = nc.dram_tensor("bq", [CW, 1], F32, kind="ExternalInput")
    bk = nc.dram_tensor("bk", [CW, 1], F32, kind="ExternalInput")
    bv = nc.dram_tensor("bv", [CW, 1], F32, kind="ExternalInput")
    Wo = nc.dram_tensor("Wo", [CW, D], BF16, kind="ExternalInput")
    OUT = nc.dram_tensor("OUT", [R, D], BF16, kind="ExternalOutput")

    with tile.TileContext(nc) as tc, ExitStack() as ctx:
        const = ctx.enter_context(tc.tile_pool(name="const", bufs=1))
        # persistent SBUF buffers, per batch to avoid false sharing
        QT = [const.tile([CW, S], BF16, tag=f"QT{b}", name=f"QT{b}")
              for b in range(B)]
        KT = [const.tile([CW, S], BF16, tag=f"KT{b}", name=f"KT{b}")
              for b in range(B)]
        ATT = [const.tile([CW, S], BF16, tag=f"ATT{b}", name=f"ATT{b}")
               for b in range(B)]
        # V' per head: [s-part(128) x k-block-pair x 2, HD cols + ones col]
        # fp8 so attn@V runs as DoubleRow (K=256 per matmul, 2 rows/cycle);
        # free dim padded 65->80 (DoubleRow LDW wants k-tile step %16 == 0)
        VP = [const.tile([128, HPC, NKB // 2, 2, 80], FP8, tag=f"VP{b}",
                         name=f"VP{b}")
              for b in range(B)]
        # all of x^T resident: [128, r-block, c-chunk, 512]
        XT = const.tile([128, NRB, NC8, 512], BF16, tag="XT")

        w_sb = {nm: const.tile([128, NC8 * CW], BF16, tag=f"w{nm}",
                               name=f"w{nm}")
                for nm in ("v", "q", "k")}
        b_sb = {nm: const.tile([CW, 1], F32, tag=f"b{nm}", name=f"b{nm}")
                for nm in ("v", "q", "k")}
        wo = const.tile([CW, D], BF16, tag="wo")
        ident = const.tile([128, 128], BF16, tag="ident")
        make_identity(nc, ident[:])  # gpsimd queue, first

        wdr = {"v": Wv, "q": Wq, "k": Wk}
        bdr = {"v": bv, "q": bq, "k": bk}
        xsrc = xT.rearrange("(c p) n -> p c n", p=128)

        # prime the ACT exp table first on the scalar queue (before its DMAs)
        actwarm = const.tile([1, 1], F32, tag="actwarm")
        warm1 = const.tile([1, 1], F32, tag="warm1")
        nc.vector.memset(warm1[:], 1.0)
        nc.scalar.activation(actwarm[:], warm1[:], AF.Exp)

        def load_x_chunk(eng, r, c):
            eng.dma_start(XT[:, r, c, :], xsrc[:, c, r * 512:(r + 1) * 512])

        def load_x_half(eng, r, c, half):
            o = half * 256
            eng.dma_start(XT[:, r, c, o:o + 256],
                          xsrc[:, c, r * 512 + o:r * 512 + o + 256])

        def load_x_block(eng, r):
            for c in range(NC8):
                load_x_chunk(eng, r, c)

        def load_w_quarter(eng, nm, jq):
            o = jq * (NC8 * CW // 4)
            eng.dma_start(w_sb[nm][:, o:o + NC8 * CW // 4],
                          wdr[nm][:, o:o + NC8 * CW // 4])

        # Startup choreography. A single dma_start moves ~20GB/s with ~2us
        # init, so the first tiles are split small (weights in quarters, the
        # first x blocks in halves) and fanned round-robin over FOUR engine
        # DMA queues so the first projection group can start by ~13us.
        wave1 = []
        for c in range(NC8):
            if c < 4:
                wave1.append(("w", "q", c))
            elif c < 8:
                wave1.append(("w", "k", c - 4))
            wave1.append(("xh", 0, c, 0))
            wave1.append(("xh", 0, c, 1))
        for j in range(4):
            wave1.append(("w", "v", j))
        for nm in ("q", "k", "v"):
            wave1.append(("b", nm))
        rings = [nc.sync, nc.gpsimd, nc.scalar]
        for k, item in enumerate(wave1):
            eng = rings[k % 3]
            if item[0] == "w":
                load_w_quarter(eng, item[1], item[2])
            elif item[0] == "xh":
                load_x_half(eng, item[1], item[2], item[3])
            else:
                eng.dma_start(b_sb[item[1]][:], bdr[item[1]][:])
        # wave 2: rest of x on the sync/gpsimd rings only (vector/scalar are
        # needed for compute from ~14us on)
        for c in range(NC8):
            load_x_half(nc.sync, 1, c, 0)
            load_x_half(nc.gpsimd, 1, c, 1)
        for c in range(NC8):
            load_x_half(nc.sync, 2, c, 0)
            load_x_half(nc.gpsimd, 2, c, 1)
        # prime the gpsimd partition_broadcast library (lib load is ~us;
        # first real broadcast is at ~35us)
        bcwarm = const.tile([2, 1], F32, tag="bcwarm")
        nc.gpsimd.partition_broadcast(bcwarm[:], warm1[:])
        load_x_block(nc.sync, 3)
        for r in (4, 5):
            load_x_block(nc.sync, r)
        nc.gpsimd.dma_start(wo[:], Wo[:])
        for r in (6, 7):
            load_x_block(nc.gpsimd, r)

        # ones columns of V' (vector queue, after its wave-1 DMA issues)
        for b in range(B):
            for h in range(HPC):
                nc.vector.memset(VP[b][:, h, :, :, HD:HD + 1], 1.0)

        vtp = ctx.enter_context(tc.tile_pool(name="vt", bufs=3))
        outp = ctx.enter_context(tc.tile_pool(name="outp", bufs=8))
        nrms = ctx.enter_context(tc.tile_pool(name="nrms", bufs=6))
        rbcp = ctx.enter_context(tc.tile_pool(name="rbc", bufs=3))
        ptp = ctx.enter_context(tc.tile_pool(name="pt", bufs=12))

        spp = ctx.enter_context(tc.tile_pool(name="sp", bufs=2, space="PSUM"))
        attp = ctx.enter_context(tc.tile_pool(name="att", bufs=2, space="PSUM"))
        scr = ctx.enter_context(tc.tile_pool(name="scr", bufs=2, space="PSUM"))

        # ---- projections ----
        def pe_warm(n):
            # dummy matmuls (ident x ident) that keep the tensor engine busy
            # while DMA paces the first r-block: the PE clock ramps to full
            # speed only after ~3us of sustained work, and idle gaps reset it
            wt = spp.tile([128, 1024], F32, tag="sp", name="warm")
            for k in range(n):
                nc.tensor.matmul(wt[:, 0:128], ident[:], ident[:],
                                 start=True, stop=True)

        def proj_mms(r, nm, c0, c1, ps, warm=0):
            for c in range(c0, c1):
                nc.tensor.matmul(ps[:], w_sb[nm][:, c * CW:(c + 1) * CW],
                                 XT[:, r, c, :],
                                 start=(c == 0), stop=(c == NC8 - 1))
                pe_warm(warm)

        def proj_finish(r, nm, ps):
            b, rb = r // (NRB // B), r % (NRB // B)
            dst = {"q": QT, "k": KT}
            if nm in dst:
                nc.vector.tensor_scalar_add(
                    dst[nm][b][:, rb * 512:(rb + 1) * 512], ps[:], b_sb[nm][:])
                return None
            vt = vtp.tile([128, 512], BF16, tag="vt", name=f"vt{r}")
            nc.vector.tensor_scalar_add(vt[:], ps[:], b_sb[nm][:])
            return vt

        def vtrans(r, vt, t_in):
            # transpose one 128-col block of vt into V' rows, both heads
            b, rb = r // (NRB // B), r % (NRB // B)
            t = rb * 4 + t_in
            tp = scr.tile([128, 128], BF16, tag="scr", name="tp")
            nc.tensor.transpose(tp[:], vt[:, t_in * 128:(t_in + 1) * 128],
                                ident[:])
            for h in range(HPC):
                nc.vector.tensor_copy(VP[b][:, h, t // 2, t % 2, 0:HD],
                                      tp[:, h * HD:(h + 1) * HD])

        def proj_block(r, weave=(), warm=0, order=("v", "q", "k")):
            weave = list(weave)       # hooks between 4-matmul segments
            for nm in order:
                ps = scr.tile([128, 512], F32, tag="scr", name=f"ps{nm}{r}")
                proj_mms(r, nm, 0, 4, ps, warm)
                if weave:
                    weave.pop(0)()
                proj_mms(r, nm, 4, NC8, ps, warm)
                vt = proj_finish(r, nm, ps)
                if weave:
                    weave.pop(0)()
                if vt is not None:
                    for t_in in range(4):
                        vtrans(r, vt, t_in)
            for w in weave:
                w()

        # ---- task queues: proj (high priority, rb-labelled) and outproj ----
        pq = deque()
        oq = deque()

        def drain(n=1):
            for _ in range(n):
                if pq:
                    pq.popleft()[1]()
                elif oq:
                    oq.popleft()()

        def drain_proj_through(rb):
            while pq and pq[0][0] <= rb:
                pq.popleft()[1]()

        def push_proj_tasks(r):
            state = {}
            for nm in ("v", "q", "k"):
                def t_a(r=r, nm=nm):
                    ps = scr.tile([128, 512], F32, tag="scr",
                                  name=f"ps{nm}{r}")
                    state[nm] = ps
                    proj_mms(r, nm, 0, 3, ps)

                def t_b(r=r, nm=nm):
                    proj_mms(r, nm, 3, 6, state[nm])

                def t_c(r=r, nm=nm):
                    proj_mms(r, nm, 6, NC8, state[nm])
                    vt = proj_finish(r, nm, state[nm])
                    if vt is not None:
                        state["vt"] = vt
                pq.append((r, t_a))
                pq.append((r, t_b))
                pq.append((r, t_c))
                if nm == "v":
                    for t0 in range(4):
                        def t_d(r=r, t0=t0):
                            vtrans(r, state["vt"], t0)
                        pq.append((r, t_d))

        def push_outproj(b, j):
            for rc in range(4):
                for oc in range(D // 512):
                    def t_o(b=b, j=j, rc=rc, oc=oc):
                        ro = j * 512 + rc * 128
                        k = rc * (D // 512) + oc
                        if k % 2:
                            po = spp.tile([128, 512], F32, tag="sp", name="po")
                        else:
                            po = scr.tile([128, 512], F32, tag="scr",
                                          name="po")
                        nc.tensor.matmul(po[:], ATT[b][:, ro:ro + 128],
                                         wo[:, oc * 512:(oc + 1) * 512],
                                         start=True, stop=True)
                        ot = outp.tile([128, 512], BF16, tag="ot", name="ot")
                        nc.vector.tensor_copy(ot[:], po[:])
                        # alternate rings: sync's DMA queue slots are the
                        # bottleneck (issues wait ~10us for ring space)
                        ring = nc.sync if k % 2 else nc.gpsimd
                        ring.dma_start(
                            OUT[b * S + ro:b * S + ro + 128,
                                oc * 512:(oc + 1) * 512], ot[:])
                    oq.append(t_o)

        # ---- global attention pipeline ----
        # global k-block index g in [0, NT): b = g//64, j = (g//16)%4, t = g%16
        pair_tiles = {}
        chunk_att = {}

        def decode(g):
            return g // (NQ * NKB), (g // NKB) % NQ, g % NKB

        def issue_scores(g):
            b, j, t = decode(g)
            if b == 1:
                drain_proj_through(NRB // B + t // 4)
            tp, sub = t // 2, t % 2
            if sub == 0:
                pair_tiles[g // 2] = ptp.tile([128, 2, HPC, 512], FP8,
                                              tag="pt", name="pt")
            sp = spp.tile([128, 1024], F32, tag="sp", name="sp")
            for h in range(HPC):
                nc.tensor.matmul(
                    sp[:, h * 512:(h + 1) * 512],
                    KT[b][h * HD:(h + 1) * HD, t * 128:(t + 1) * 128],
                    QT[b][h * HD:(h + 1) * HD, j * 512:(j + 1) * 512],
                    start=True, stop=True)
            nc.scalar.activation(pair_tiles[g // 2][:, sub, :, :], sp[:],
                                 AF.Exp, scale=0.125)

        def issue_attnv(i):  # pair index i in [0, NT//2)
            b, j, t = decode(2 * i)
            tp = t // 2
            if tp == 0:
                chunk_att[(b, j)] = [
                    attp.tile([80, 512], F32, tag="att",
                              name=f"att{b}_{j}_{h}") for h in range(HPC)]
            att = chunk_att[(b, j)]
            for h in range(HPC):
                nc.tensor.matmul(att[h][:], VP[b][:, h, tp, :, :],
                                 pair_tiles[i][:, :, h, :],
                                 perf_mode=DRM,
                                 start=(tp == 0), stop=(tp == NKB // 2 - 1))
            del pair_tiles[i]

        def issue_norm(b, j):
            # pipeline the two heads' chains so gpsimd broadcast(h0) overlaps
            # the DVE reciprocal(h1)
            att = chunk_att.pop((b, j))
            rrows, rbcs = [], []
            for h in range(HPC):
                srow = nrms.tile([1, 512], F32, tag="srow", name="srow")
                nc.vector.tensor_copy(srow[:], att[h][HD:HD + 1, :])
                rrow = nrms.tile([1, 512], F32, tag="rrow", name="rrow")
                nc.vector.reciprocal_approx_fast(out=rrow[:], in_=srow[:])
                rrows.append(rrow)
            for h in range(HPC):
                rbc = rbcp.tile([HD, 512], F32, tag="rbc", name="rbc")
                nc.gpsimd.partition_broadcast(rbc[:], rrows[h][:])
                rbcs.append(rbc)
            for h in range(HPC):
                nc.vector.tensor_mul(
                    ATT[b][h * HD:(h + 1) * HD, j * 512:(j + 1) * 512],
                    att[h][0:HD, :], rbcs[h][:])
            push_outproj(b, j)

        # ---------------- prologue: b0 projections + chunk-0 woven in --------
        pe_warm(20)
        # r0: scores t0-t3 woven right after K(r0) lands. The first scores
        # after a same-block K bias-add needs a ~1.5us PE delay (pe_warm
        # junk) or its LDWEIGHTS reads the KT slice before the DVE write
        # lands (observed NaN otherwise); the junk also keeps the PE clock
        # ramped through the DMA-paced region.
        weave0 = [lambda: None] * 3

        def w_g0():
            pe_warm(14)
            issue_scores(0)
        weave0.append(w_g0)
        for g in (1, 2):
            def w_s0(g=g):
                issue_scores(g)
            weave0.append(w_s0)
        proj_block(0, weave0, warm=2, order=("q", "k", "v"))
        issue_scores(3)
        cursor = 4
        # r1-r3: ~6 exps per block — qc0's next k-blocks (guarded: same-block
        # KT write) interleaved with qc1 scores (old KT rows, QT from the
        # prior block) so the scalar engine stays fed through the prologue
        sched = {
            1: [None, (14, 4), (0, 5), (0, 6), (0, 7), (8, 16), (0, 17)],
            2: [(0, 18), (14, 8), (0, 9), (0, 10), (0, 11), (0, 19)],
            3: [(0, 20), (14, 12), (0, 13), (0, 14), (0, 15), (0, 21),
                (0, 22), (0, 23)],
        }
        for r in (1, 2, 3):
            weave = []
            for item in sched[r]:
                if item is None:
                    weave.append(lambda: None)
                else:
                    def w_s(item=item):
                        if item[0]:
                            pe_warm(item[0])
                        issue_scores(item[1])
                    weave.append(w_s)
            proj_block(r, weave, order=("k", "q", "v"))
            for i in range(2 * (r - 1), 2 * r):
                issue_attnv(i)
        cursor = 24
        while cursor < 28:   # prime the steady pipeline (old deps only)
            issue_scores(cursor)
            cursor += 1

        for r in range(NRB // B, NRB):
            push_proj_tasks(r)

        # ---------------- steady state ----------------
        for i in range(6, NT // 2):
            while cursor < min(2 * i + 12, NT):
                issue_scores(cursor)
                cursor += 1
            issue_attnv(i)
            drain(2)
            if i % (NKB // 2) == NKB // 2 - 1:
                b, j, _ = decode(2 * i)
                issue_norm(b, j)
        while pq or oq:
            drain(1)
    nc.finalize()
    return nc


_nc_cache = None


def _get_nc():
    global _nc_cache
    if _nc_cache is None:
        _nc_cache = build()
    return _nc_cache


def kernel(x, Wq, bq, Wk, bk, Wv, bv, Wo, bo):
    x = np.asarray(x, dtype=np.float32)
    xTf = np.ascontiguousarray(x.reshape(R, D).T).astype(ml_dtypes.bfloat16)

    def wshard(W, sl):
        # [D, CW] slice -> partition-major [128, NC8*CW] contiguous
        w = np.asarray(W, np.float32)[:, sl]
        w = w.reshape(NC8, 128, CW).transpose(1, 0, 2).reshape(128, NC8 * CW)
        return np.ascontiguousarray(w).astype(ml_dtypes.bfloat16)

    in_maps = []
    for i in range(NCORES):
        sl = slice(i * CW, (i + 1) * CW)
        in_maps.append({
            "xT": xTf,
            "Wq": wshard(Wq, sl),
            "Wk": wshard(Wk, sl),
            "Wv": wshard(Wv, sl),
            "bq": np.ascontiguousarray(np.asarray(bq, np.float32)[sl]).reshape(CW, 1),
            "bk": np.ascontiguousarray(np.asarray(bk, np.float32)[sl]).reshape(CW, 1),
            "bv": np.ascontiguousarray(np.asarray(bv, np.float32)[sl]).reshape(CW, 1),
            "Wo": np.ascontiguousarray(
                np.asarray(Wo, np.float32)[sl, :]).astype(ml_dtypes.bfloat16),
        })
    nc = _get_nc()
    trace = bool(int(os.environ.get("KERNEL_TRACE", "0")))
    res = run_bass_kernel_spmd(nc, in_maps, core_ids=list(range(NCORES)),
                               trace=trace)
    if trace and res.exec_time_ns is not None:
        print(f"HW exec time: {res.exec_time_ns} ns")
        print(f"mean exec time: {res.mean_exec_time_ns} ns")
        if res.instructions_and_trace is not None:
            print("trace:", res.instructions_and_trace[1])
    acc = np.zeros((R, D), dtype=np.float64)
    for r_ in res.results:
        acc += np.asarray(r_["OUT"]).astype(np.float64)
    acc += np.asarray(bo, np.float32).astype(np.float64)[None, :]
    return acc.reshape(B, S, D).astype(np.float32)


# revision 54
# speedup vs baseline: 1.0369x; 1.0369x over previous
"""Multi-head attention (B=2, S=2048, H=16, HD=64, D=1024) on 8 trn2 cores.

Sharding: 2 heads per core (tensor-parallel over heads). Each core computes
its heads' Q/K/V projections (column-sharded weights), full attention for its
4 (batch, head) pairs, and a partial output projection (row-sharded Wo).
Host sums the 8 partials and adds bo.

The scalar engine's exp is the hard floor (1 elem/cycle/partition ->
~143us/core for the 16.8M scores), so the kernel is built as one continuous
exp pipeline: a global scores->exp cursor runs 2 k-blocks ahead across chunk
boundaries, attn@V consumes exp pairs as fp8 DoubleRow matmuls (K=256,
2 rows/cycle), and all projection / output-projection work is drained as
micro-tasks in the per-iteration PE slack. Batch-0 chunk-0 attention is woven
into the projection prologue so exp starts as early as possible.
"""
import os
from collections import deque
from contextlib import ExitStack

import numpy as np
import ml_dtypes

import concourse.bass as bass
import concourse.tile as tile
import concourse.mybir as mybir
from concourse import bacc
from concourse.bass_utils import run_bass_kernel_spmd
from concourse.masks import make_identity

B, S, D = 2, 2048, 1024
H, HD = 16, 64
NCORES = 8
HPC = H // NCORES          # heads per core = 2
CW = HPC * HD              # column width per core = 128
R = B * S                  # total rows = 4096
NKB = S // 128             # k-blocks per (b,h) = 16
NQ = S // 512              # q-chunks per batch = 4
NC8 = D // 128             # d_in chunks = 8
NRB = R // 512             # 512-row projection blocks = 8
NT = B * NQ * NKB          # global k-block count = 128

F32 = mybir.dt.float32
BF16 = mybir.dt.bfloat16
FP8 = mybir.dt.float8e4
DRM = mybir.MatmulPerfMode.DoubleRow
AF = mybir.ActivationFunctionType


def build():
    nc = bacc.Bacc("TRN2", target_bir_lowering=False, debug=False)
    xT = nc.dram_tensor("xT", [D, R], BF16, kind="ExternalInput")
    # weights pre-transposed on host to [128, NC8*CW] (partition-major)
    Wq = nc.dram_tensor("Wq", [128, NC8 * CW], BF16, kind="ExternalInput")
    Wk = nc.dram_tensor("Wk", [128, NC8 * CW], BF16, kind="ExternalInput")
    Wv = nc.dram_tensor("Wv", [128, NC8 * CW], BF16, kind="ExternalInput")
    bq = nc.dram_tensor("bq", [CW, 1], F32, kind="ExternalInput")
    bk = nc.dram_tensor("bk", [CW, 1], F32, kind="ExternalInput")
    bv = nc.dram_tensor("bv", [CW, 1], F32, kind="ExternalInput")
    Wo = nc.dram_tensor("Wo", [CW, D], BF16, kind="ExternalInput")
    OUT = nc.dram_tensor("OUT", [R, D], BF16, kind="ExternalOutput")

    with tile.TileContext(nc) as tc, ExitStack() as ctx:
        const = ctx.enter_context(tc.tile_pool(name="const", bufs=1))
        # persistent SBUF buffers, per batch to avoid false sharing
        QT = [const.tile([CW, S], BF16, tag=f"QT{b}", name=f"QT{b}")
              for b in range(B)]
        KT = [const.tile([CW, S], BF16, tag=f"KT{b}", name=f"KT{b}")
              for b in range(B)]
        ATT = [const.tile([CW, S], BF16, tag=f"ATT{b}", name=f"ATT{b}")
               for b in range(B)]
        # V' per head: [s-part(128) x k-block-pair x 2, HD cols + ones col]
        # fp8 so attn@V runs as DoubleRow (K=256 per matmul, 2 rows/cycle);
        # free dim padded 65->80 (DoubleRow LDW wants k-tile step %16 == 0)
        VP = [const.tile([128, HPC, NKB // 2, 2, 80], FP8, tag=f"VP{b}",
                         name=f"VP{b}")
              for b in range(B)]
        # all of x^T resident: [128, r-block, c-chunk, 512]
        XT = const.tile([128, NRB, NC8, 512], BF16, tag="XT")

        w_sb = {nm: const.tile([128, NC8 * CW], BF16, tag=f"w{nm}",
                               name=f"w{nm}")
                for nm in ("v", "q", "k")}
        b_sb = {nm: const.tile([CW, 1], F32, tag=f"b{nm}", name=f"b{nm}")
                for nm in ("v", "q", "k")}
        wo = const.tile([CW, D], BF16, tag="wo")
        ident = const.tile([128, 128], BF16, tag="ident")
        make_identity(nc, ident[:])  # gpsimd queue, first

        wdr = {"v": Wv, "q": Wq, "k": Wk}
        bdr = {"v": bv, "q": bq, "k": bk}
        xsrc = xT.rearrange("(c p) n -> p c n", p=128)

        # prime the ACT exp table first on the scalar queue (before its DMAs)
        actwarm = const.tile([1, 1], F32, tag="actwarm")
        warm1 = const.tile([1, 1], F32, tag="warm1")
        nc.vector.memset(warm1[:], 1.0)
        nc.scalar.activation(actwarm[:], warm1[:], AF.Exp)

        def load_x_chunk(eng, r, c):
            eng.dma_start(XT[:, r, c, :], xsrc[:, c, r * 512:(r + 1) * 512])

        def load_x_half(eng, r, c, half):
            o = half * 256
            eng.dma_start(XT[:, r, c, o:o + 256],
                          xsrc[:, c, r * 512 + o:r * 512 + o + 256])

        def load_x_block(eng, r):
            for c in range(NC8):
                load_x_chunk(eng, r, c)

        def load_w_quarter(eng, nm, jq):
            o = jq * (NC8 * CW // 4)
            eng.dma_start(w_sb[nm][:, o:o + NC8 * CW // 4],
                          wdr[nm][:, o:o + NC8 * CW // 4])

        # Startup choreography. A single dma_start moves ~20GB/s with ~2us
        # init, so the first tiles are split small (weights in quarters, the
        # first x blocks in halves) and fanned round-robin over FOUR engine
        # DMA queues so the first projection group can start by ~13us.
        wave1 = []
        for c in range(NC8):
            if c < 4:
                wave1.append(("w", "q", c))
            elif c < 8:
                wave1.append(("w", "k", c - 4))
            wave1.append(("xh", 0, c, 0))
            wave1.append(("xh", 0, c, 1))
        for j in range(4):
            wave1.append(("w", "v", j))
        for nm in ("q", "k", "v"):
            wave1.append(("b", nm))
        rings = [nc.sync, nc.gpsimd, nc.scalar]
        for k, item in enumerate(wave1):
            eng = rings[k % 3]
            if item[0] == "w":
                load_w_quarter(eng, item[1], item[2])
            elif item[0] == "xh":
                load_x_half(eng, item[1], item[2], item[3])
            else:
                eng.dma_start(b_sb[item[1]][:], bdr[item[1]][:])
        # wave 2: rest of x on the sync/gpsimd rings only (vector/scalar are
        # needed for compute from ~14us on)
        for c in range(NC8):
            load_x_half(nc.sync, 1, c, 0)
            load_x_half(nc.gpsimd, 1, c, 1)
        for c in range(NC8):
            load_x_half(nc.sync, 2, c, 0)
            load_x_half(nc.gpsimd, 2, c, 1)
        # prime the gpsimd partition_broadcast library (lib load is ~us;
        # first real broadcast is at ~35us)
        bcwarm = const.tile([2, 1], F32, tag="bcwarm")
        nc.gpsimd.partition_broadcast(bcwarm[:], warm1[:])
        load_x_block(nc.sync, 3)
        for r in (4, 5):
            load_x_block(nc.sync, r)
        nc.gpsimd.dma_start(wo[:], Wo[:])
        for r in (6, 7):
            load_x_block(nc.gpsimd, r)

        # ones columns of V' (vector queue, after its wave-1 DMA issues)
        for b in range(B):
            for h in range(HPC):
                nc.vector.memset(VP[b][:, h, :, :, HD:HD + 1], 1.0)

        vtp = ctx.enter_context(tc.tile_pool(name="vt", bufs=3))
        outp = ctx.enter_context(tc.tile_pool(name="outp", bufs=8))
        nrms = ctx.enter_context(tc.tile_pool(name="nrms", bufs=6))
        rbcp = ctx.enter_context(tc.tile_pool(name="rbc", bufs=3))
        ptp = ctx.enter_context(tc.tile_pool(name="pt", bufs=12))

        spp = ctx.enter_context(tc.tile_pool(name="sp", bufs=2, space="PSUM"))
        attp = ctx.enter_context(tc.tile_pool(name="att", bufs=2, space="PSUM"))
        scr = ctx.enter_context(tc.tile_pool(name="scr", bufs=2, space="PSUM"))

        # ---- projections ----
        def pe_warm(n):
            # dummy matmuls (ident x ident) that keep the tensor engine busy
            # while DMA paces the first r-block: the PE clock ramps to full
            # speed only after ~3us of sustained work, and idle gaps reset it
            wt = spp.tile([128, 1024], F32, tag="sp", name="warm")
            for k in range(n):
                nc.tensor.matmul(wt[:, 0:128], ident[:], ident[:],
                                 start=True, stop=True)

        def proj_mms(r, nm, c0, c1, ps, warm=0):
            for c in range(c0, c1):
                nc.tensor.matmul(ps[:], w_sb[nm][:, c * CW:(c + 1) * CW],
                                 XT[:, r, c, :],
                                 start=(c == 0), stop=(c == NC8 - 1))
                pe_warm(warm)

        def proj_finish(r, nm, ps):
            b, rb = r // (NRB // B), r % (NRB // B)
            dst = {"q": QT, "k": KT}
            if nm in dst:
                nc.vector.tensor_scalar_add(
                    dst[nm][b][:, rb * 512:(rb + 1) * 512], ps[:], b_sb[nm][:])
                return None
            vt = vtp.tile([128, 512], BF16, tag="vt", name=f"vt{r}")
            nc.vector.tensor_scalar_add(vt[:], ps[:], b_sb[nm][:])
            return vt

        def vtrans(r, vt, t_in):
            # transpose one 128-col block of vt into V' rows, both heads
            b, rb = r // (NRB // B), r % (NRB // B)
            t = rb * 4 + t_in
            tp = scr.tile([128, 128], BF16, tag="scr", name="tp")
            nc.tensor.transpose(tp[:], vt[:, t_in * 128:(t_in + 1) * 128],
                                ident[:])
            for h in range(HPC):
                nc.vector.tensor_copy(VP[b][:, h, t // 2, t % 2, 0:HD],
                                      tp[:, h * HD:(h + 1) * HD])

        def proj_block(r, weave=(), warm=0, order=("v", "q", "k")):
            weave = list(weave)       # hooks between 4-matmul segments
            for nm in order:
                ps = scr.tile([128, 512], F32, tag="scr", name=f"ps{nm}{r}")
                proj_mms(r, nm, 0, 4, ps, warm)
                if weave:
                    weave.pop(0)()
                proj_mms(r, nm, 4, NC8, ps, warm)
                vt = proj_finish(r, nm, ps)
                if weave:
                    weave.pop(0)()
                if vt is not None:
                    for t_in in range(4):
                        vtrans(r, vt, t_in)
            for w in weave:
                w()

        # ---- task queues: proj (high priority, rb-labelled) and outproj ----
        pq = deque()
        oq = deque()

        def drain(n=1):
            for _ in range(n):
                if pq:
                    pq.popleft()[1]()
                elif oq:
                    oq.popleft()()

        def drain_proj_through(rb):
            while pq and pq[0][0] <= rb:
                pq.popleft()[1]()

        def push_proj_tasks(r):
            state = {}
            for nm in ("v", "q", "k"):
                def t_a(r=r, nm=nm):
                    ps = scr.tile([128, 512], F32, tag="scr",
                                  name=f"ps{nm}{r}")
                    state[nm] = ps
                    proj_mms(r, nm, 0, 3, ps)

                def t_b(r=r, nm=nm):
                    proj_mms(r, nm, 3, 6, state[nm])

                def t_c(r=r, nm=nm):
                    proj_mms(r, nm, 6, NC8, state[nm])
                    vt = proj_finish(r, nm, state[nm])
                    if vt is not None:
                        state["vt"] = vt
                pq.append((r, t_a))
                pq.append((r, t_b))
                pq.append((r, t_c))
                if nm == "v":
                    for t0 in range(4):
                        def t_d(r=r, t0=t0):
                            vtrans(r, state["vt"], t0)
                        pq.append((r, t_d))

        def push_outproj(b, j):
            for rc in range(4):
                for oc in range(D // 512):
                    def t_o(b=b, j=j, rc=rc, oc=oc):
                        ro = j * 512 + rc * 128
                        k = rc * (D // 512) + oc
                        if k % 2:
                            po = spp.tile([128, 512], F32, tag="sp", name="po")
                        else:
                            po = scr.tile([128, 512], F32, tag="scr",
                                          name="po")
                        nc.tensor.matmul(po[:], ATT[b][:, ro:ro + 128],
                                         wo[:, oc * 512:(oc + 1) * 512],
                                         start=True, stop=True)
                        ot = outp.tile([128, 512], BF16, tag="ot", name="ot")
                        nc.vector.tensor_copy(ot[:], po[:])
                        nc.sync.dma_start(
                            OUT[b * S + ro:b * S + ro + 128,
                                oc * 512:(oc + 1) * 512], ot[:])
                    oq.append(t_o)

        # ---- global attention pipeline ----
        # global k-block index g in [0, NT): b = g//64, j = (g//16)%4, t = g%16
        pair_tiles = {}
        chunk_att = {}

        def decode(g):
            return g // (NQ * NKB), (g // NKB) % NQ, g % NKB

        def issue_scores(g):
            b, j, t = decode(g)
            if b == 1:
                drain_proj_through(NRB // B + t // 4)
            tp, sub = t // 2, t % 2
            if sub == 0:
                pair_tiles[g // 2] = ptp.tile([128, 2, HPC, 512], FP8,
                                              tag="pt", name="pt")
            sp = spp.tile([128, 1024], F32, tag="sp", name="sp")
            for h in range(HPC):
                nc.tensor.matmul(
                    sp[:, h * 512:(h + 1) * 512],
                    KT[b][h * HD:(h + 1) * HD, t * 128:(t + 1) * 128],
                    QT[b][h * HD:(h + 1) * HD, j * 512:(j + 1) * 512],
                    start=True, stop=True)
            nc.scalar.activation(pair_tiles[g // 2][:, sub, :, :], sp[:],
                                 AF.Exp, scale=0.125)

        def issue_attnv(i):  # pair index i in [0, NT//2)
            b, j, t = decode(2 * i)
            tp = t // 2
            if tp == 0:
                chunk_att[(b, j)] = [
                    attp.tile([80, 512], F32, tag="att",
                              name=f"att{b}_{j}_{h}") for h in range(HPC)]
            att = chunk_att[(b, j)]
            for h in range(HPC):
                nc.tensor.matmul(att[h][:], VP[b][:, h, tp, :, :],
                                 pair_tiles[i][:, :, h, :],
                                 perf_mode=DRM,
                                 start=(tp == 0), stop=(tp == NKB // 2 - 1))
            del pair_tiles[i]

        def issue_norm(b, j):
            # pipeline the two heads' chains so gpsimd broadcast(h0) overlaps
            # the DVE reciprocal(h1)
            att = chunk_att.pop((b, j))
            rrows, rbcs = [], []
            for h in range(HPC):
                srow = nrms.tile([1, 512], F32, tag="srow", name="srow")
                nc.vector.tensor_copy(srow[:], att[h][HD:HD + 1, :])
                rrow = nrms.tile([1, 512], F32, tag="rrow", name="rrow")
                nc.vector.reciprocal_approx_fast(out=rrow[:], in_=srow[:])
                rrows.append(rrow)
            for h in range(HPC):
                rbc = rbcp.tile([HD, 512], F32, tag="rbc", name="rbc")
                nc.gpsimd.partition_broadcast(rbc[:], rrows[h][:])
                rbcs.append(rbc)
            for h in range(HPC):
                nc.vector.tensor_mul(
                    ATT[b][h * HD:(h + 1) * HD, j * 512:(j + 1) * 512],
                    att[h][0:HD, :], rbcs[h][:])
            push_outproj(b, j)

        # ---------------- prologue: b0 projections + chunk-0 woven in --------
        pe_warm(20)
        # r0: scores t0-t3 woven right after K(r0) lands. The first scores
        # after a same-block K bias-add needs a ~1.5us PE delay (pe_warm
        # junk) or its LDWEIGHTS reads the KT slice before the DVE write
        # lands (observed NaN otherwise); the junk also keeps the PE clock
        # ramped through the DMA-paced region.
        weave0 = [lambda: None] * 3

        def w_g0():
            pe_warm(14)
            issue_scores(0)
        weave0.append(w_g0)
        for g in (1, 2):
            def w_s0(g=g):
                issue_scores(g)
            weave0.append(w_s0)
        proj_block(0, weave0, warm=2, order=("q", "k", "v"))
        issue_scores(3)
        cursor = 4
        # r1-r3: ~6 exps per block — qc0's next k-blocks (guarded: same-block
        # KT write) interleaved with qc1 scores (old KT rows, QT from the
        # prior block) so the scalar engine stays fed through the prologue
        sched = {
            1: [None, (14, 4), (0, 5), (0, 6), (0, 7), (8, 16), (0, 17)],
            2: [(0, 18), (14, 8), (0, 9), (0, 10), (0, 11), (0, 19)],
            3: [(0, 20), (14, 12), (0, 13), (0, 14), (0, 15), (0, 21),
                (0, 22), (0, 23)],
        }
        for r in (1, 2, 3):
            weave = []
            for item in sched[r]:
                if item is None:
                    weave.append(lambda: None)
                else:
                    def w_s(item=item):
                        if item[0]:
                            pe_warm(item[0])
                        issue_scores(item[1])
                    weave.append(w_s)
            proj_block(r, weave, order=("k", "q", "v"))
            for i in range(2 * (r - 1), 2 * r):
                issue_attnv(i)
        cursor = 24
        while cursor < 28:   # prime the steady pipeline (old deps only)
            issue_scores(cursor)
            cursor += 1

        for r in range(NRB // B, NRB):
            push_proj_tasks(r)

        # ---------------- steady state ----------------
        for i in range(6, NT // 2):
            while cursor < min(2 * i + 12, NT):
                issue_scores(cursor)
                cursor += 1
            issue_attnv(i)
            drain(2)
            if i % (NKB // 2) == NKB // 2 - 1:
                b, j, _ = decode(2 * i)
                issue_norm(b, j)
        while pq or oq:
            drain(1)
    nc.finalize()
    return nc


_nc_cache = None


def _get_nc():
    global _nc_cache
    if _nc_cache is None:
        _nc_cache = build()
    return _nc_cache


def kernel(x, Wq, bq, Wk, bk, Wv, bv, Wo, bo):
    x = np.asarray(x, dtype=np.float32)
    xTf = np.ascontiguousarray(x.reshape(R, D).T).astype(ml_dtypes.bfloat16)

    def wshard(W, sl):
        # [D, CW] slice -> partition-major [128, NC8*CW] contiguous
        w = np.asarray(W, np.float32)[:, sl]
        w = w.reshape(NC8, 128, CW).transpose(1, 0, 2).reshape(128, NC8 * CW)
        return np.ascontiguousarray(w).astype(ml_dtypes.bfloat16)

    in_maps = []
    for i in range(NCORES):
        sl = slice(i * CW, (i + 1) * CW)
        in_maps.append({
            "xT": xTf,
            "Wq": wshard(Wq, sl),
            "Wk": wshard(Wk, sl),
            "Wv": wshard(Wv, sl),
            "bq": np.ascontiguousarray(np.asarray(bq, np.float32)[sl]).reshape(CW, 1),
            "bk": np.ascontiguousarray(np.asarray(bk, np.float32)[sl]).reshape(CW, 1),
            "bv": np.ascontiguousarray(np.asarray(bv, np.float32)[sl]).reshape(CW, 1),
            "Wo": np.ascontiguousarray(
                np.asarray(Wo, np.float32)[sl, :]).astype(ml_dtypes.bfloat16),
        })
    nc = _get_nc()
    trace = bool(int(os.environ.get("KERNEL_TRACE", "0")))
    res = run_bass_kernel_spmd(nc, in_maps, core_ids=list(range(NCORES)),
                               trace=trace)
    if trace and res.exec_time_ns is not None:
        print(f"HW exec time: {res.exec_time_ns} ns")
        print(f"mean exec time: {res.mean_exec_time_ns} ns")
        if res.instructions_and_trace is not None:
            print("trace:", res.instructions_and_trace[1])
    acc = np.zeros((R, D), dtype=np.float64)
    for r_ in res.results:
        acc += np.asarray(r_["OUT"]).astype(np.float64)
    acc += np.asarray(bo, np.float32).astype(np.float64)[None, :]
    return acc.reshape(B, S, D).astype(np.float32)
